# revision 9
# baseline (speedup 1.0000x reference)
"""Trainium2 Bass kernel for nn_BinaryTreeTopDownLSTM.

Math notes (from the reference):
  - The top-down traversal gives BOTH children the same parent state and
    composer() has no left/right distinction, so every node at a given level
    of a tree is identical.  The whole internal traversal collapses to a
    10-step recurrence on a per-tree [M] state.
  - Of the 6 output feature chunks, ce/he depend on embs (per-leaf); cph,
    cpc, hph, hpc are per-tree constants broadcast over all 2048 leaves.

Sharding: data-parallel over trees, 8 trees per core on 8 cores.

Layout: leaves are mapped p-major — SBUF partition p holds leaves
[16p, 16p+16) of a tree, so every DRAM<->SBUF transfer is 128 descriptors
of 8-12KB contiguous bytes.  ce/he are computed per "group" of 4 leaf
sub-tiles to amortize per-instruction overhead (esp. ACT).
"""

import sys

sys.path.insert(0, "/opt/trn_rl_repo")

import numpy as np

B, L, M = 64, 2048, 128
NCORES = 8
S = B // NCORES  # trees per core
P = 128          # partitions
T = L // P       # leaf sub-tiles per tree (16)
G = 4            # sub-tiles per compute group
F = 6 * M        # output features (768)
DEPTH = 11       # log2(L)

_CACHE = {}


def _build(with_bias: bool):
    """Builds + compiles the per-core Bass module (same program on all cores)."""
    import concourse.bacc as bacc
    import concourse.bass as bass
    import concourse.mybir as mybir
    import concourse.tile as tile
    from concourse.masks import make_identity

    fp32 = mybir.dt.float32
    AF = mybir.ActivationFunctionType

    nc = bacc.Bacc("TRN2", target_bir_lowering=False, debug=False)

    embs = nc.dram_tensor("embs", [S, L, M], fp32, kind="ExternalInput").ap()
    rooth = nc.dram_tensor("root_h", [S, M], fp32, kind="ExternalInput").ap()
    rootc = nc.dram_tensor("root_c", [S, M], fp32, kind="ExternalInput").ap()
    wap = {
        n: nc.dram_tensor(n, [M, M], fp32, kind="ExternalInput").ap()
        for n in ("Wi", "Wf", "Wu", "Wc", "Wo")
    }
    bap = {}
    if with_bias:
        bap = {
            n: nc.dram_tensor(n, [M], fp32, kind="ExternalInput").ap()
            for n in ("bi", "bf", "bu", "bc", "bo")
        }
    out = nc.dram_tensor("out", [S, L, F], fp32, kind="ExternalOutput").ap()

    # p-major leaf tiling: partition p <-> leaves [T*p, T*p+T)
    embs_r = embs.rearrange("s (p t) m -> s p t m", t=T)  # [S, 128, T, M]
    out_r = out.rearrange("s (p t) f -> s p t f", t=T)    # [S, 128, T, F]

    with tile.TileContext(nc) as tc:
        with (
            tc.tile_pool(name="consts", bufs=1) as consts,
            tc.tile_pool(name="state", bufs=2) as state,
            tc.tile_pool(name="tmp", bufs=3) as tmp,
            tc.tile_pool(name="xin", bufs=4) as xin,
            tc.tile_pool(name="obuf", bufs=2) as obuf,
            tc.tile_pool(name="ps_pro", bufs=1, space="PSUM") as ps_pro,
            tc.tile_pool(name="ps_xt", bufs=2, space="PSUM") as ps_xt,
            tc.tile_pool(name="ps_mm", bufs=2, space="PSUM") as ps_mm,
            tc.tile_pool(name="dram", bufs=1, space="DRAM") as dram,
        ):
            # ---------------- constants ----------------
            ident = consts.tile([P, P], fp32)
            make_identity(nc, ident)

            w = {}
            for n in ("Wi", "Wf", "Wu"):
                w[n] = consts.tile([P, P], fp32, name=f"w_{n}")
                nc.gpsimd.dma_start(out=w[n], in_=wap[n])
            w_co = consts.tile([P, 2 * M], fp32)  # [Wc | Wo]
            nc.gpsimd.dma_start(out=w_co[:, 0:M], in_=wap["Wc"])
            nc.gpsimd.dma_start(out=w_co[:, M : 2 * M], in_=wap["Wo"])

            bias = {}
            brow = {}
            if with_bias:
                for n in ("bi", "bf", "bu", "bc", "bo"):
                    src = bap[n]
                    bias[n] = consts.tile([P, 1], fp32, name=f"b_{n}")
                    nc.gpsimd.dma_start(
                        out=bias[n],
                        in_=bass.AP(
                            tensor=src.tensor, offset=src.offset,
                            ap=[src.ap[0], [1, 1]],
                        ),
                    )
                for n in ("bc", "bo"):
                    # bias replicated on every partition (features on free dim)
                    src = bap[n]
                    brow[n] = consts.tile([P, M], fp32, name=f"br_{n}")
                    nc.gpsimd.dma_start(
                        out=brow[n],
                        in_=bass.AP(
                            tensor=src.tensor, offset=src.offset,
                            ap=[[0, P], src.ap[0]],
                        ),
                    )

            def act(out_ap, in_ap, func, bname=None):
                kw = {}
                if with_bias and bname is not None:
                    kw["bias"] = bias[bname][:, 0:1]
                    if func == AF.Copy:
                        func = AF.Identity
                nc.scalar.activation(out_ap, in_ap, func, **kw)

            # -------- prefetch all embs loads (SP ring, ahead of stores) -------
            xbs = []
            for s in range(S):
                xb = xin.tile([P, T, M], fp32, tag="xb")
                nc.sync.dma_start(out=xb, in_=embs_r[s])
                xbs.append(xb)

            # ---------------- root state (transposed: [feat, tree]) -----------
            r_sb = tmp.tile([S, 2, M], fp32)
            nc.sync.dma_start(out=r_sb[:, 0, :], in_=rooth)
            nc.sync.dma_start(out=r_sb[:, 1, :], in_=rootc)
            hc_ps = ps_pro.tile([P, 2 * S], fp32, tag="pro")
            nc.tensor.transpose(hc_ps[:, 0:S], r_sb[:, 0, :], ident[:S, :S])
            nc.tensor.transpose(hc_ps[:, S : 2 * S], r_sb[:, 1, :], ident[:S, :S])
            hc = state.tile([P, 2 * S], fp32)  # [:, 0:S]=h^T  [:, S:2S]=c^T
            nc.vector.tensor_copy(hc, hc_ps)

            # ---------------- 10 composer levels ----------------
            for _lvl in range(1, DEPTH):
                g_ps = ps_pro.tile([P, 3 * S], fp32, tag="pro")
                nc.tensor.matmul(g_ps[:, 0:S], w["Wi"], hc[:, 0:S], start=True, stop=True)
                nc.tensor.matmul(g_ps[:, S : 2 * S], w["Wf"], hc[:, 0:S], start=True, stop=True)
                nc.tensor.matmul(g_ps[:, 2 * S : 3 * S], w["Wu"], hc[:, 0:S], start=True, stop=True)
                gs = tmp.tile([P, 3 * S], fp32)
                act(gs[:, 0:S], g_ps[:, 0:S], AF.Sigmoid, "bi")
                act(gs[:, S : 2 * S], g_ps[:, S : 2 * S], AF.Sigmoid, "bf")
                act(gs[:, 2 * S : 3 * S], g_ps[:, 2 * S : 3 * S], AF.Tanh, "bu")
                iu = tmp.tile([P, S], fp32)
                nc.vector.tensor_mul(iu, gs[:, 0:S], gs[:, 2 * S : 3 * S])
                fc = tmp.tile([P, S], fp32)
                nc.vector.tensor_mul(fc, gs[:, S : 2 * S], hc[:, S : 2 * S])
                hc_new = state.tile([P, 2 * S], fp32, tag="hc")
                nc.vector.tensor_add(hc_new[:, S : 2 * S], iu, fc)
                act(hc_new[:, 0:S], hc_new[:, S : 2 * S], AF.Tanh)
                hc = hc_new

            # ---------------- leaf transform of parent h/c ----------------
            lt_ps = ps_pro.tile([P, 4 * S], fp32, tag="pro")
            nc.tensor.matmul(lt_ps[:, 0 : 2 * S], w_co[:, 0:M], hc[:, 0 : 2 * S], start=True, stop=True)
            nc.tensor.matmul(lt_ps[:, 2 * S : 4 * S], w_co[:, M : 2 * M], hc[:, 0 : 2 * S], start=True, stop=True)
            cc = tmp.tile([P, 2 * S], fp32)   # [cph^T | cpc^T]
            act(cc, lt_ps[:, 0 : 2 * S], AF.Copy, "bc")
            th = tmp.tile([P, 2 * S], fp32)
            act(th, lt_ps[:, 0 : 2 * S], AF.Tanh, "bc")
            sg = tmp.tile([P, 2 * S], fp32)
            act(sg, lt_ps[:, 2 * S : 4 * S], AF.Sigmoid, "bo")
            hh = tmp.tile([P, 2 * S], fp32)   # [hph^T | hpc^T]
            nc.vector.tensor_mul(hh, sg, th)

            # ---------------- per-tree broadcast rows ----------------
            # bsm rows: tree s -> [cph | cpc | hph | hpc] (512 features)
            bs_ps = ps_pro.tile([S, 4 * M], fp32, tag="pro")
            nc.tensor.transpose(bs_ps[:, 0:M], cc[:, 0:S], ident)
            nc.tensor.transpose(bs_ps[:, M : 2 * M], cc[:, S : 2 * S], ident)
            nc.tensor.transpose(bs_ps[:, 2 * M : 3 * M], hh[:, 0:S], ident)
            nc.tensor.transpose(bs_ps[:, 3 * M : 4 * M], hh[:, S : 2 * S], ident)
            bsm = tmp.tile([S, 4 * M], fp32)
            nc.vector.tensor_copy(bsm, bs_ps)
            scr = dram.tile([S, 4 * M], fp32)
            nc.sync.dma_start(out=scr, in_=bsm)
            # broadcast [S, 512] across all 128 partitions
            bcast = consts.tile([P, S, 4 * M], fp32)
            nc.gpsimd.dma_start(
                out=bcast,
                in_=bass.AP(
                    tensor=scr.tensor, offset=scr.offset,
                    ap=[[0, P], scr.ap[0], scr.ap[1]],
                ),
            )

            # ---------------- main loop: ce / he over embs ----------------
            # ob holds full output rows [t, 768]; every engine write into it is
            # a single contiguous run per partition (multi-run writes are ~30x
            # slower on DVE/GpSimd).  One 6MB store per tree => 128 descriptors
            # of 12KB, near line-rate.
            for s in range(S):
                xb = xbs[s]
                ob = obuf.tile([P, T, F], fp32)
                for g in range(T // G):
                    t0 = g * G
                    xT_ps = ps_xt.tile([P, G, M], fp32, tag="xT")
                    for j in range(G):
                        nc.tensor.transpose(xT_ps[:, j, :], xb[:, t0 + j, :], ident)
                    xT = tmp.tile([P, G, M], fp32, tag="xT_sb")
                    nc.vector.tensor_copy(xT, xT_ps)
                    mm_ps = ps_mm.tile([P, G, 2 * M], fp32, tag="mm")
                    for j in range(G):
                        nc.tensor.matmul(
                            mm_ps[:, j, :], xT[:, j, :], w_co, start=True, stop=True
                        )
                    tct = tmp.tile([P, G * M], fp32, tag="tct")
                    sot = tmp.tile([P, G * M], fp32, tag="sot")
                    if with_bias:
                        # per-feature bias lives on the free dim here: add the
                        # partition-replicated bias rows on DVE, then activate.
                        osum = tmp.tile([P, G, M], fp32, tag="osum")
                        for j in range(G):
                            nc.vector.tensor_add(
                                ob[:, t0 + j, 0:M], mm_ps[:, j, 0:M], brow["bc"]
                            )
                            nc.vector.tensor_add(
                                osum[:, j, :], mm_ps[:, j, M : 2 * M], brow["bo"]
                            )
                        nc.scalar.activation(tct, ob[:, t0 : t0 + G, 0:M], AF.Tanh)
                        nc.scalar.activation(sot, osum, AF.Sigmoid)
                    else:
                        # batched transcendentals (strided psum read, packed write)
                        nc.scalar.activation(tct, mm_ps[:, :, 0:M], AF.Tanh)
                        nc.scalar.activation(sot, mm_ps[:, :, M : 2 * M], AF.Sigmoid)
                        for j in range(G):
                            # ce: single-run copy psum -> ob; split DVE/ACT
                            if j % 2 == 0:
                                nc.vector.tensor_copy(ob[:, t0 + j, 0:M], mm_ps[:, j, 0:M])
                            else:
                                nc.scalar.copy(ob[:, t0 + j, 0:M], mm_ps[:, j, 0:M])
                    for j in range(G):
                        t = t0 + j
                        # he = sigmoid(o) * tanh(ce)  (DVE, single-run write)
                        nc.vector.tensor_mul(
                            ob[:, t, 3 * M : 4 * M],
                            sot[:, j * M : (j + 1) * M],
                            tct[:, j * M : (j + 1) * M],
                        )
                        # broadcast chunk fills (single-run 1KB copies)
                        nc.gpsimd.tensor_copy(
                            ob[:, t, M : 3 * M], bcast[:, s, 0 : 2 * M]
                        )
                        if j % 2 == 0:
                            nc.gpsimd.tensor_copy(
                                ob[:, t, 4 * M : 6 * M], bcast[:, s, 2 * M : 4 * M]
                            )
                        else:
                            nc.vector.tensor_copy(
                                ob[:, t, 4 * M : 6 * M], bcast[:, s, 2 * M : 4 * M]
                            )

                # one 6MB store per tree (SP ring, behind the prefetched loads)
                nc.sync.dma_start(out=out_r[s], in_=ob)

    nc.compile()
    return nc


def _get_nc(with_bias: bool):
    key = ("nc", with_bias)
    if key not in _CACHE:
        _CACHE[key] = _build(with_bias)
    return _CACHE[key]


RUN_KWARGS = {}  # dev harness may inject e.g. tmpdir for traces


def run(inputs, trace=False):
    """Returns (full_output [B, L, 6M], exec_time_ns or None)."""
    from concourse import bass_utils

    inputs = {k: np.ascontiguousarray(np.asarray(v), dtype=np.float32) for k, v in inputs.items()}
    with_bias = any(
        np.any(inputs[n]) for n in ("bi", "bf", "bu", "bc", "bo") if n in inputs
    )
    nc = _get_nc(with_bias)

    in_maps = []
    for c in range(NCORES):
        sl = slice(c * S, (c + 1) * S)
        m = {
            "embs": inputs["embs"][sl],
            "root_h": inputs["root_h"][sl],
            "root_c": inputs["root_c"][sl],
            "Wi": inputs["Wi"], "Wf": inputs["Wf"], "Wu": inputs["Wu"],
            "Wc": inputs["Wc"], "Wo": inputs["Wo"],
        }
        if with_bias:
            for n in ("bi", "bf", "bu", "bc", "bo"):
                m[n] = inputs[n]
        in_maps.append(m)

    res = bass_utils.run_bass_kernel_spmd(
        nc, in_maps, core_ids=list(range(NCORES)), trace=trace, **RUN_KWARGS
    )
    full = np.concatenate([np.asarray(r["out"]) for r in res.results], axis=0)
    return full, res.exec_time_ns


def kernel(**inputs) -> np.ndarray:
    out, _ = run(inputs, trace=False)
    return out


# revision 11
# speedup vs baseline: 1.2633x; 1.2633x over previous
"""Trainium2 Bass kernel for nn_BinaryTreeTopDownLSTM.

Math notes (from the reference):
  - The top-down traversal gives BOTH children the same parent state and
    composer() has no left/right distinction, so every node at a given level
    of a tree is identical.  The whole internal traversal collapses to a
    10-step recurrence on a per-tree [M] state.
  - Of the 6 output feature chunks, ce/he depend on embs (per-leaf); cph,
    cpc, hph, hpc are per-tree constants broadcast over all 2048 leaves.

Sharding: data-parallel over trees, 8 trees per core on 8 cores.

Layout: leaves are mapped p-major — SBUF partition p holds leaves
[16p, 16p+16) of a tree, so every DRAM<->SBUF transfer is 128 descriptors
of 8-12KB contiguous bytes.  ce/he are computed per "group" of 4 leaf
sub-tiles to amortize per-instruction overhead (esp. ACT).
"""

import sys

sys.path.insert(0, "/opt/trn_rl_repo")

import numpy as np

B, L, M = 64, 2048, 128
NCORES = 8
S = B // NCORES  # trees per core
P = 128          # partitions
T = L // P       # leaf sub-tiles per tree (16)
G = 4            # sub-tiles per compute group
F = 6 * M        # output features (768)
DEPTH = 11       # log2(L)

_CACHE = {}


def _build(with_bias: bool):
    """Builds + compiles the per-core Bass module (same program on all cores)."""
    import concourse.bacc as bacc
    import concourse.bass as bass
    import concourse.mybir as mybir
    import concourse.tile as tile
    from concourse.masks import make_identity

    fp32 = mybir.dt.float32
    AF = mybir.ActivationFunctionType

    nc = bacc.Bacc("TRN2", target_bir_lowering=False, debug=False)

    embs = nc.dram_tensor("embs", [S, L, M], fp32, kind="ExternalInput").ap()
    rooth = nc.dram_tensor("root_h", [S, M], fp32, kind="ExternalInput").ap()
    rootc = nc.dram_tensor("root_c", [S, M], fp32, kind="ExternalInput").ap()
    wap = {
        n: nc.dram_tensor(n, [M, M], fp32, kind="ExternalInput").ap()
        for n in ("Wi", "Wf", "Wu", "Wc", "Wo")
    }
    bap = {}
    if with_bias:
        bap = {
            n: nc.dram_tensor(n, [M], fp32, kind="ExternalInput").ap()
            for n in ("bi", "bf", "bu", "bc", "bo")
        }
    out = nc.dram_tensor("out", [S, L, F], fp32, kind="ExternalOutput").ap()

    # p-major leaf tiling: partition p <-> leaves [T*p, T*p+T)
    embs_r = embs.rearrange("s (p t) m -> s p t m", t=T)  # [S, 128, T, M]
    out_r = out.rearrange("s (p t) f -> s p t f", t=T)    # [S, 128, T, F]

    with tile.TileContext(nc) as tc:
        with (
            tc.tile_pool(name="consts", bufs=1) as consts,
            tc.tile_pool(name="state", bufs=2) as state,
            tc.tile_pool(name="tmp", bufs=3) as tmp,
            tc.tile_pool(name="xin", bufs=4) as xin,
            tc.tile_pool(name="obuf", bufs=2) as obuf,
            tc.tile_pool(name="ps_pro", bufs=1, space="PSUM") as ps_pro,
            tc.tile_pool(name="ps_xt", bufs=2, space="PSUM") as ps_xt,
            tc.tile_pool(name="ps_mm", bufs=2, space="PSUM") as ps_mm,
            tc.tile_pool(name="dram", bufs=1, space="DRAM") as dram,
        ):
            # ---------------- constants ----------------
            ident = consts.tile([P, P], fp32)
            make_identity(nc, ident)

            w = {}
            for n in ("Wi", "Wf", "Wu"):
                w[n] = consts.tile([P, P], fp32, name=f"w_{n}")
                nc.gpsimd.dma_start(out=w[n], in_=wap[n])
            w_co = consts.tile([P, 2 * M], fp32)  # [Wc | Wo]
            nc.gpsimd.dma_start(out=w_co[:, 0:M], in_=wap["Wc"])
            nc.gpsimd.dma_start(out=w_co[:, M : 2 * M], in_=wap["Wo"])

            bias = {}
            brow = {}
            if with_bias:
                for n in ("bi", "bf", "bu", "bc", "bo"):
                    src = bap[n]
                    bias[n] = consts.tile([P, 1], fp32, name=f"b_{n}")
                    nc.gpsimd.dma_start(
                        out=bias[n],
                        in_=bass.AP(
                            tensor=src.tensor, offset=src.offset,
                            ap=[src.ap[0], [1, 1]],
                        ),
                    )
                for n in ("bc", "bo"):
                    # bias replicated on every partition (features on free dim)
                    src = bap[n]
                    brow[n] = consts.tile([P, M], fp32, name=f"br_{n}")
                    nc.gpsimd.dma_start(
                        out=brow[n],
                        in_=bass.AP(
                            tensor=src.tensor, offset=src.offset,
                            ap=[[0, P], src.ap[0]],
                        ),
                    )

            def act(out_ap, in_ap, func, bname=None):
                kw = {}
                if with_bias and bname is not None:
                    kw["bias"] = bias[bname][:, 0:1]
                    if func == AF.Copy:
                        func = AF.Identity
                nc.scalar.activation(out_ap, in_ap, func, **kw)

            # -------- prefetch all embs loads (SP ring, ahead of stores) -------
            xbs = []
            for s in range(S):
                xb = xin.tile([P, T, M], fp32, tag="xb")
                nc.sync.dma_start(out=xb, in_=embs_r[s])
                xbs.append(xb)

            # ---------------- root state (transposed: [feat, tree]) -----------
            r_sb = tmp.tile([S, 2, M], fp32)
            nc.sync.dma_start(out=r_sb[:, 0, :], in_=rooth)
            nc.sync.dma_start(out=r_sb[:, 1, :], in_=rootc)
            hc_ps = ps_pro.tile([P, 2 * S], fp32, tag="pro")
            nc.tensor.transpose(hc_ps[:, 0:S], r_sb[:, 0, :], ident[:S, :S])
            nc.tensor.transpose(hc_ps[:, S : 2 * S], r_sb[:, 1, :], ident[:S, :S])
            hc = state.tile([P, 2 * S], fp32)  # [:, 0:S]=h^T  [:, S:2S]=c^T
            nc.vector.tensor_copy(hc, hc_ps)

            # ---------------- 10 composer levels ----------------
            for _lvl in range(1, DEPTH):
                g_ps = ps_pro.tile([P, 3 * S], fp32, tag="pro")
                nc.tensor.matmul(g_ps[:, 0:S], w["Wi"], hc[:, 0:S], start=True, stop=True)
                nc.tensor.matmul(g_ps[:, S : 2 * S], w["Wf"], hc[:, 0:S], start=True, stop=True)
                nc.tensor.matmul(g_ps[:, 2 * S : 3 * S], w["Wu"], hc[:, 0:S], start=True, stop=True)
                gs = tmp.tile([P, 3 * S], fp32)
                act(gs[:, 0:S], g_ps[:, 0:S], AF.Sigmoid, "bi")
                act(gs[:, S : 2 * S], g_ps[:, S : 2 * S], AF.Sigmoid, "bf")
                act(gs[:, 2 * S : 3 * S], g_ps[:, 2 * S : 3 * S], AF.Tanh, "bu")
                iu = tmp.tile([P, S], fp32)
                nc.vector.tensor_mul(iu, gs[:, 0:S], gs[:, 2 * S : 3 * S])
                fc = tmp.tile([P, S], fp32)
                nc.vector.tensor_mul(fc, gs[:, S : 2 * S], hc[:, S : 2 * S])
                hc_new = state.tile([P, 2 * S], fp32, tag="hc")
                nc.vector.tensor_add(hc_new[:, S : 2 * S], iu, fc)
                act(hc_new[:, 0:S], hc_new[:, S : 2 * S], AF.Tanh)
                hc = hc_new

            # ---------------- leaf transform of parent h/c ----------------
            lt_ps = ps_pro.tile([P, 4 * S], fp32, tag="pro")
            nc.tensor.matmul(lt_ps[:, 0 : 2 * S], w_co[:, 0:M], hc[:, 0 : 2 * S], start=True, stop=True)
            nc.tensor.matmul(lt_ps[:, 2 * S : 4 * S], w_co[:, M : 2 * M], hc[:, 0 : 2 * S], start=True, stop=True)
            cc = tmp.tile([P, 2 * S], fp32)   # [cph^T | cpc^T]
            act(cc, lt_ps[:, 0 : 2 * S], AF.Copy, "bc")
            th = tmp.tile([P, 2 * S], fp32)
            act(th, lt_ps[:, 0 : 2 * S], AF.Tanh, "bc")
            sg = tmp.tile([P, 2 * S], fp32)
            act(sg, lt_ps[:, 2 * S : 4 * S], AF.Sigmoid, "bo")
            hh = tmp.tile([P, 2 * S], fp32)   # [hph^T | hpc^T]
            nc.vector.tensor_mul(hh, sg, th)

            # ---------------- per-tree broadcast rows ----------------
            # bsm rows: tree s -> [cph | cpc | hph | hpc] (512 features)
            bs_ps = ps_pro.tile([S, 4 * M], fp32, tag="pro")
            nc.tensor.transpose(bs_ps[:, 0:M], cc[:, 0:S], ident)
            nc.tensor.transpose(bs_ps[:, M : 2 * M], cc[:, S : 2 * S], ident)
            nc.tensor.transpose(bs_ps[:, 2 * M : 3 * M], hh[:, 0:S], ident)
            nc.tensor.transpose(bs_ps[:, 3 * M : 4 * M], hh[:, S : 2 * S], ident)
            bsm = tmp.tile([S, 4 * M], fp32)
            nc.vector.tensor_copy(bsm, bs_ps)
            scr = dram.tile([S, 4 * M], fp32)
            nc.sync.dma_start(out=scr, in_=bsm)
            # broadcast [S, 512] across all 128 partitions
            bcast = consts.tile([P, S, 4 * M], fp32)
            nc.gpsimd.dma_start(
                out=bcast,
                in_=bass.AP(
                    tensor=scr.tensor, offset=scr.offset,
                    ap=[[0, P], scr.ap[0], scr.ap[1]],
                ),
            )

            # ------- broadcast-chunk stores: depend only on the prologue -------
            # 32MB of the 48MB output is per-tree constants; stream them from
            # t~15us so the DMA engines are never starved while compute ramps.
            for s in range(S):
                ov = out_r[s]
                bsrc = bcast[:, s, :]
                for k, (src_off, c0) in enumerate(((0, M), (2 * M, 4 * M))):
                    rep = bass.AP(
                        tensor=bsrc.tensor, offset=bsrc.offset + src_off,
                        ap=[bsrc.ap[0], [0, T], [1, 2 * M]],
                    )
                    eng = nc.scalar if (s + k) % 2 == 0 else nc.gpsimd
                    eng.dma_start(out=ov[:, :, c0 : c0 + 2 * M], in_=rep)

            # ---------------- main loop: ce / he over embs ----------------
            # Engine writes into ob are single contiguous runs per partition
            # (multi-run strided writes are ~30x slower on DVE/GpSimd); the
            # store DMA descriptors do the feature interleave.
            for s in range(S):
                xb = xbs[s]
                ob = obuf.tile([P, T, 2 * M], fp32)  # [:, t, 0:M]=ce [:, t, M:2M]=he
                for g in range(T // G):
                    t0 = g * G
                    xT_ps = ps_xt.tile([P, G, M], fp32, tag="xT")
                    for j in range(G):
                        nc.tensor.transpose(xT_ps[:, j, :], xb[:, t0 + j, :], ident)
                    xT = tmp.tile([P, G, M], fp32, tag="xT_sb")
                    nc.vector.tensor_copy(xT, xT_ps)
                    mm_ps = ps_mm.tile([P, G, 2 * M], fp32, tag="mm")
                    for j in range(G):
                        nc.tensor.matmul(
                            mm_ps[:, j, :], xT[:, j, :], w_co, start=True, stop=True
                        )
                    tct = tmp.tile([P, G * M], fp32, tag="tct")
                    sot = tmp.tile([P, G * M], fp32, tag="sot")
                    if with_bias:
                        # per-feature bias lives on the free dim here: add the
                        # partition-replicated bias rows on DVE, then activate.
                        osum = tmp.tile([P, G, M], fp32, tag="osum")
                        for j in range(G):
                            nc.vector.tensor_add(
                                ob[:, t0 + j, 0:M], mm_ps[:, j, 0:M], brow["bc"]
                            )
                            nc.vector.tensor_add(
                                osum[:, j, :], mm_ps[:, j, M : 2 * M], brow["bo"]
                            )
                        nc.scalar.activation(tct, ob[:, t0 : t0 + G, 0:M], AF.Tanh)
                        nc.scalar.activation(sot, osum, AF.Sigmoid)
                    else:
                        # batched transcendentals (strided psum read, packed write)
                        nc.scalar.activation(tct, mm_ps[:, :, 0:M], AF.Tanh)
                        nc.scalar.activation(sot, mm_ps[:, :, M : 2 * M], AF.Sigmoid)
                        for j in range(G):
                            # ce: single-run copy psum -> ob  (DVE)
                            nc.vector.tensor_copy(ob[:, t0 + j, 0:M], mm_ps[:, j, 0:M])
                    for j in range(G):
                        # he = sigmoid(o) * tanh(ce)  (GpSimd, single-run write)
                        nc.gpsimd.tensor_mul(
                            ob[:, t0 + j, M : 2 * M],
                            sot[:, j * M : (j + 1) * M],
                            tct[:, j * M : (j + 1) * M],
                        )
                    # ce/he stores per half tree, issued as soon as ready
                    if g % 2 == 1:
                        th = slice(t0 + G - T // 2, t0 + G)
                        ov = out_r[s]
                        nc.sync.dma_start(out=ov[:, th, 0:M], in_=ob[:, th, 0:M])
                        nc.sync.dma_start(
                            out=ov[:, th, 3 * M : 4 * M], in_=ob[:, th, M : 2 * M]
                        )

    nc.compile()
    return nc


def _get_nc(with_bias: bool):
    key = ("nc", with_bias)
    if key not in _CACHE:
        _CACHE[key] = _build(with_bias)
    return _CACHE[key]


RUN_KWARGS = {}  # dev harness may inject e.g. tmpdir for traces


def run(inputs, trace=False):
    """Returns (full_output [B, L, 6M], exec_time_ns or None)."""
    from concourse import bass_utils

    inputs = {k: np.ascontiguousarray(np.asarray(v), dtype=np.float32) for k, v in inputs.items()}
    with_bias = any(
        np.any(inputs[n]) for n in ("bi", "bf", "bu", "bc", "bo") if n in inputs
    )
    nc = _get_nc(with_bias)

    in_maps = []
    for c in range(NCORES):
        sl = slice(c * S, (c + 1) * S)
        m = {
            "embs": inputs["embs"][sl],
            "root_h": inputs["root_h"][sl],
            "root_c": inputs["root_c"][sl],
            "Wi": inputs["Wi"], "Wf": inputs["Wf"], "Wu": inputs["Wu"],
            "Wc": inputs["Wc"], "Wo": inputs["Wo"],
        }
        if with_bias:
            for n in ("bi", "bf", "bu", "bc", "bo"):
                m[n] = inputs[n]
        in_maps.append(m)

    res = bass_utils.run_bass_kernel_spmd(
        nc, in_maps, core_ids=list(range(NCORES)), trace=trace, **RUN_KWARGS
    )
    full = np.concatenate([np.asarray(r["out"]) for r in res.results], axis=0)
    return full, res.exec_time_ns


def kernel(**inputs) -> np.ndarray:
    out, _ = run(inputs, trace=False)
    return out
